# revision 25
# baseline (speedup 1.0000x reference)
"""Single-head causal attention on 8 TRN2 NeuronCores — v4 (merged stream).

Sharding: 2 cores per batch element (B=4); core parity p owns interleaved
128-row t-blocks {2j+p}. Host swaps adjacent xT column blocks on odd cores
so one SPMD program serves all cores (masks differ as data).

v4 (trace-driven evolution of v2/v3):
  - ONE merged flash stream: PSUM strip A runs groups 0 then 3, strip B
    runs 1 then 2 (staggered one round) -> 20/20 rounds per strip instead
    of v2's 12/28 two-pass split; exp (ACT, the serial floor at ~41us
    busy) streams end-to-end without the inter-pass epilogue gap.
  - acc PSUM tiles ping-pong: acc1 holds g0(rows 0:33)+g1(64:97), acc2
    holds g3+g2; epilogues for g0/g1 run as mid-stream fillers, and tail
    slots start as soon as their causal span is accumulated.
  - tail epilogue normalize-muls run on the then-idle ACT engine.
  - causal mask applied multiplicatively (0/1 bf16) on the GPSIMD engine
    to expt in SBUF -- off the DVE (v2 added -200 on PSUM via DVE).
  - prologue projection drains on the (idle until first exp) ACT engine.
  - small operands host-packed into one contiguous [128, 1184] bf16
    tensor (v2's rearranged wqkv DMA was a 1536-descriptor gather).
  - k/v upper half projected at N=512 half granularity.
  - v3's XBAR dma transposes reverted: 1.2us engine occupancy each and
    in-order DMA queues starved attnV of vones -> PE identity-matmul
    transposes (v2 scheme) with no queue dependency.
  - Output stored bf16; host casts to fp32 and folds bv@Wp + bp.
"""

import math
import sys

for _p in ("/opt/trn_rl_repo", "/opt/trn_rl_repo/concourse"):
    if _p not in sys.path:
        sys.path.insert(0, _p)

import ml_dtypes
import numpy as np

BF16 = ml_dtypes.bfloat16

B, T, D, H = 4, 4096, 512, 32
NSLOT = 16          # 128-row t-blocks per core
NSB = T // 128      # 32 s-blocks
NG = 4              # slot groups of 4 slots (512 t-cols)
SCALE = 1.0 / math.sqrt(32.0)
PACKW = 3 * 4 * H + D + 2 * 128 + H + 2 * 128   # wqkv|wp|mask01|ident|maskadd

_CACHE = {}


def build_nc(debug=False):
    import concourse.mybir as mybir
    import concourse.tile as tile
    from concourse import bacc

    dt = mybir.dt
    Act = mybir.ActivationFunctionType
    nc = bacc.Bacc("TRN2", target_bir_lowering=False, debug=False)

    xT = nc.dram_tensor("xT", [4, 128, T], dt.bfloat16, kind="ExternalInput").ap()
    pack = nc.dram_tensor(
        "pack", [128, PACKW], dt.bfloat16, kind="ExternalInput"
    ).ap()
    bqk = nc.dram_tensor("bqk", [128, 2], dt.float32, kind="ExternalInput").ap()
    out = nc.dram_tensor(
        "out", [NSLOT * 128, D], dt.bfloat16, kind="ExternalOutput"
    ).ap()

    with tile.TileContext(nc) as tc, tc.tile_pool(
        name="singles", bufs=1
    ) as singles, tc.tile_pool(name="exp_pool", bufs=8) as exp_pool, tc.tile_pool(
        name="attnT_pool", bufs=4
    ) as attnT_pool, tc.tile_pool(
        name="recip_pool", bufs=4
    ) as recip_pool, tc.tile_pool(name="out_pool", bufs=3) as out_pool:
        # ---- resident SBUF tensors -----------------------------------
        xT_sb = singles.tile([128, 4, T], dt.bfloat16)
        pack_sb = singles.tile([128, PACKW], dt.bfloat16)
        bqk_sb = singles.tile([128, 2], dt.float32)
        bq_sb = bqk_sb[:, 0:1]
        bk_sb = bqk_sb[:, 1:2]
        wqkv_sb = pack_sb[:, 0 : 3 * 4 * H].rearrange(
            "p (w c h) -> p w c h", w=3, c=4
        )
        wq_sb = wqkv_sb[:, 0, :, :]
        wk_sb = wqkv_sb[:, 1, :, :]
        wv_sb = wqkv_sb[:, 2, :, :]
        wp_sb = pack_sb[:, 3 * 4 * H : 3 * 4 * H + D]
        mask_sb = pack_sb[:, 3 * 4 * H + D : 3 * 4 * H + D + 256].rearrange(
            "p (m j) -> p m j", m=2
        )
        ident_sb = pack_sb[:, 3 * 4 * H + D + 256 : 3 * 4 * H + D + 256 + H]
        maskadd_sb = pack_sb[:, 3 * 4 * H + D + 256 + H :].rearrange(
            "p (m j) -> p m j", m=2
        )
        # strip layouts: partition 32c+h holds s-block 4*q8+c, col j
        kT4 = singles.tile([128, 8, 128], dt.bfloat16)
        vT4 = singles.tile([128, 8, 128], dt.bfloat16)
        # qT replicated on all 4 strips: partition 32r+h = q[t, h]
        qT4 = singles.tile([128, NG, 512], dt.bfloat16)
        vones = singles.tile([128, NSB, H + 1], dt.bfloat16)
        ones1 = singles.tile([128, 1], dt.bfloat16)

        # ---- input DMA ------------------------------------------------
        # sync + scalar HWDGE queues; every scalar-queue DMA issue steals
        # ~630ns of ACT sequencer time, so the scalar queue carries only
        # pre-stream pieces and drains before the first exp.
        q0 = slice(0, T // 4)
        # wq first as a tiny transfer: the very first matmul needs only
        # wq + the first x piece; the rest of the pack follows
        nc.sync.dma_start(out=pack_sb[:, 0 : 4 * H], in_=pack[:, 0 : 4 * H])
        nc.scalar.dma_start(out=bqk_sb, in_=bqk)
        nc.sync.dma_start(out=xT_sb[:, 0, q0], in_=xT[0, :, q0])
        nc.scalar.dma_start(out=xT_sb[:, 1, q0], in_=xT[1, :, q0])
        nc.sync.dma_start(out=pack_sb[:, 4 * H :], in_=pack[:, 4 * H :])
        nc.sync.dma_start(out=xT_sb[:, 2, q0], in_=xT[2, :, q0])
        nc.scalar.dma_start(out=xT_sb[:, 3, q0], in_=xT[3, :, q0])
        # remaining quarters, q3 before q2 (group 3's qT needs it by k=1)
        for tq in (1, 3, 2):
            tsl = slice(tq * (T // 4), (tq + 1) * (T // 4))
            nc.sync.dma_start(
                out=xT_sb[:, 0:2, tsl],
                in_=xT[0:2, :, tsl].rearrange("e p t -> p e t"),
            )
            nc.scalar.dma_start(
                out=xT_sb[:, 2:4, tsl],
                in_=xT[2:4, :, tsl].rearrange("e p t -> p e t"),
            )

        nc.gpsimd.memset(vones, 1.0)
        nc.gpsimd.memset(ones1, 1.0)

        # views: raw col = 512*q8 + 128*f + j
        xk = xT_sb.rearrange("p e (q f j) -> p e q f j", f=4, j=128)
        xq = xT_sb.rearrange("p e (g two b) -> p e g two b", two=2, b=128)

        # ---- PSUM pools (16KB/partition budget, bank = 2KB) ----------
        # ps_sc: 2 x [128,2,512]f32 = 8KB; ps_acc: 2 x [128,512]f32 = 4KB;
        # ps_util tag p: 2 x [128,512]f32 = 4KB (shared by proj/epi/vtrans)
        with tc.tile_pool(
            name="ps_sc", bufs=2, space="PSUM"
        ) as ps_sc, tc.tile_pool(
            name="ps_acc", bufs=2, space="PSUM"
        ) as ps_acc, tc.tile_pool(name="ps_util", bufs=2, space="PSUM") as ps_util:

            # ---- projection emitters ---------------------------------
            def qT_chunk(tcq, nstrip=4):
                qps = ps_util.tile([128, 512], dt.float32, name="qps", tag="p")
                for e in range(4):
                    for r in range(nstrip):
                        nc.tensor.matmul(
                            qps[32 * r : 32 * r + 32, :],
                            wq_sb[:, e, :],
                            xq[:, e, 4 * tcq : 4 * tcq + 4, 0, :],
                            start=(e == 0),
                            stop=(e == 3),
                            skip_group_check=True,
                            tile_position=(0, 32 * r),
                        )
                if nstrip == 4:
                    if tcq == 0:
                        # prologue: drain on the still-idle ACT engine
                        nc.scalar.activation(
                            qT4[:, tcq, :], qps, Act.Identity, bias=bq_sb
                        )
                    else:
                        nc.vector.tensor_scalar_add(qT4[:, tcq, :], qps, bq_sb)
                else:
                    nc.vector.tensor_scalar_add(
                        qT4[0:32, tcq, :], qps[0:32, :], bq_sb[0:32, :]
                    )
                    for r in range(1, 4):
                        nc.sync.dma_start(
                            out=qT4[32 * r : 32 * r + 32, tcq, :],
                            in_=qT4[0:32, tcq, :],
                        )

            kv_live = {}

            def kv_part(which, qlo, qhi, es):
                # one accumulation half of a k/v projection chunk; the two
                # halves share a PSUM tile via kv_live (no other tag-p
                # allocation may intervene between them)
                w_sb = wk_sb if which == "k" else wv_sb
                dst = kT4 if which == "k" else vT4
                n = (qhi - qlo) * 128
                key = (which, qlo, qhi)
                if es[0] == 0:
                    kv_live[key] = ps_util.tile(
                        [128, n], dt.float32, name="kvps", tag="p"
                    )
                ps = kv_live[key]
                for e in es:
                    for c in range(4):
                        nc.tensor.matmul(
                            ps[32 * c : 32 * c + 32, :],
                            w_sb[:, e, :],
                            xk[:, e, qlo:qhi, c, :],
                            start=(e == 0),
                            stop=(e == 3),
                            skip_group_check=True,
                            tile_position=(0, 32 * c),
                        )
                if es[-1] == 3:
                    view = ps.rearrange("p (f j) -> p f j", j=128)
                    if qlo == 0:
                        # prologue: drain on the still-idle ACT engine
                        if which == "k":
                            nc.scalar.activation(
                                dst[:, qlo:qhi, :], view, Act.Identity,
                                bias=bk_sb,
                            )
                        else:
                            nc.scalar.activation(
                                dst[:, qlo:qhi, :], view, Act.Copy
                            )
                    elif which == "k":
                        nc.vector.tensor_scalar_add(
                            dst[:, qlo:qhi, :], view, bk_sb
                        )
                    else:
                        nc.vector.tensor_copy(dst[:, qlo:qhi, :], view)
                    del kv_live[key]

            def kv_proj(which, qlo, qhi):
                kv_part(which, qlo, qhi, (0, 1, 2, 3))

            def v_trans(sb):
                # v[s,H] = vT strip slice transposed via identity matmul
                q8, c = divmod(sb, 4)
                tp = ps_util.tile([128, H], dt.float32, name="tp", tag="p")
                nc.tensor.matmul(
                    tp,
                    vT4[32 * c : 32 * c + 32, q8, :],
                    ident_sb[32 * c : 32 * c + 32, :],
                    start=True,
                    stop=True,
                    tile_position=(32 * c, 0),
                )
                nc.vector.tensor_copy(vones[:, sb, 0:H], tp)

            def vt_slice(lo, hi):
                def f():
                    for sb in range(lo, hi):
                        v_trans(sb)
                return f

            # ---- flash stream machinery ------------------------------
            def emit_scores(g, R, dve_mask=False):
                band = R >= 4 * g
                i0 = 128 * (R - 4 * g) if band else 0
                scps = ps_sc.tile([128, 2, 512], dt.float32, name="scps", tag="sc")
                for i in range(2):
                    sb = 2 * R + i
                    nc.tensor.matmul(
                        scps[:, i, i0:512],
                        kT4[32 * (sb % 4) : 32 * (sb % 4) + 32, sb // 4, :],
                        qT4[32 * (sb % 4) : 32 * (sb % 4) + 32, g, i0:512],
                        start=True,
                        stop=True,
                        tile_position=(32 * (sb % 4), 0),
                    )
                if band and dve_mask:
                    # endgame: additive -200 mask on DVE before exp; the
                    # post-exp gpsimd hop lengthens the (by then latency-
                    # bound) score->exp->attnV round trip
                    nc.vector.tensor_add(
                        scps[:, :, i0 : i0 + 128],
                        scps[:, :, i0 : i0 + 128],
                        maskadd_sb,
                    )
                expt = exp_pool.tile(
                    [128, 2, 512], dt.bfloat16, name="expt", tag="e"
                )
                nc.scalar.activation(
                    expt[:, :, i0:512],
                    scps[:, :, i0:512],
                    Act.Exp,
                    scale=SCALE,
                )
                if band and not dve_mask:
                    # multiplicative 0/1 causal mask on the idle GPSIMD
                    # engine (SBUF bf16; gpsimd cannot touch PSUM)
                    nc.gpsimd.tensor_mul(
                        expt[:, :, i0 : i0 + 128],
                        expt[:, :, i0 : i0 + 128],
                        mask_sb,
                    )
                return (g, R, expt, i0)

            def acc_of(g):
                return acc1 if g in (0, 1) else acc2

            def p64_of(g):
                return 0 if g in (0, 3) else 64

            def emit_items(items):
                # interleave the attnVs of up to two items (strip A + strip
                # B -> alternating PSUM column strips -> 2-way concurrency)
                for i in range(2):
                    for g, R, expt, i0 in items:
                        sb = 2 * R + i
                        p64 = p64_of(g)
                        nc.tensor.matmul(
                            acc_of(g)[p64 : p64 + 33, i0:512],
                            vones[:, sb, :],
                            expt[:, i, i0:512],
                            start=(sb == 0),
                            stop=(sb == 8 * g + 7),
                            skip_group_check=True,
                            tile_position=(0, p64),
                        )

            def epi_slot(g, i, copy_act=False, mul_act=False):
                p64 = p64_of(g)
                acc = acc_of(g)
                tsl = slice(i * 128, (i + 1) * 128)
                attnT = attnT_pool.tile([128, 128], dt.bfloat16, name="attnT")
                if copy_act:
                    nc.scalar.activation(
                        attnT[p64 : p64 + 33, :], acc[p64 : p64 + 33, tsl],
                        Act.Copy,
                    )
                else:
                    nc.vector.tensor_copy(
                        attnT[p64 : p64 + 33, :], acc[p64 : p64 + 33, tsl]
                    )
                dps = ps_util.tile([128, 1], dt.float32, name="dps", tag="p")
                nc.tensor.matmul(
                    dps,
                    attnT[p64 + 32 : p64 + 33, :],
                    ones1[p64 + 32 : p64 + 33, :],
                    start=True,
                    stop=True,
                    tile_position=(p64 + 32, 0),
                )
                recip = recip_pool.tile([128, 1], dt.float32, name="recip")
                nc.vector.reciprocal(recip, dps)
                ops = ps_util.tile([128, D], dt.float32, name="ops", tag="p")
                nc.tensor.matmul(
                    ops,
                    attnT[p64 : p64 + 32, :],
                    wp_sb[p64 : p64 + 32, :],
                    start=True,
                    stop=True,
                    tile_position=(p64, 0),
                )
                o1 = out_pool.tile([128, D], dt.bfloat16, name="o1")
                if mul_act:
                    # after the last exp the ACT engine is idle
                    nc.scalar.activation(o1, ops, Act.Copy, scale=recip)
                else:
                    nc.vector.tensor_scalar_mul(o1, ops, recip)
                j = 4 * g + i
                nc.sync.dma_start(out=out[j * 128 : (j + 1) * 128, :], in_=o1)

            # ---- prologue --------------------------------------------
            qT_chunk(0)
            kv_proj("k", 0, 2)
            kv_proj("v", 0, 2)
            vt_slice(0, 3)()

            acc1 = ps_acc.tile([128, 512], dt.float32, name="acc", tag="acc")
            acc2 = ps_acc.tile([128, 512], dt.float32, name="acc", tag="acc")

            # ---- merged stream: strip A = g0 then g3, B = g1 then g2 --
            A_items = [(0, R) for R in range(4)] + [(3, R) for R in range(16)]
            B_items = [(1, R) for R in range(8)] + [(2, R) for R in range(12)]

            # filler ordering is deadlock-sensitive: in-order PE means a
            # filler producing data for an ALREADY-EMITTED matmul can never
            # run; and an epi filler for (g,i) must come strictly after the
            # pop of round (g, 4g+i). fillers[k] runs at loop index k.
            def qT_dma(tcq):
                def f():
                    qT_chunk(tcq, nstrip=1)
                return f

            fillers = [
                lambda: qT_chunk(1),                     # k0; g1 scores at k=1
                lambda: qT_chunk(3),                     # k1; g3 scores at k=4
                vt_slice(3, 8),                          # k2; pops sb<8 by k>=3
                lambda: kv_part("k", 2, 4, (0, 1)),      # k3; sb 8+ scores k=5
                lambda: kv_part("k", 2, 4, (2, 3)),      # k4
                lambda: kv_part("v", 2, 4, (0, 1)),      # k5
                lambda: (                                # k6; sb8 pops at k=7
                    kv_part("v", 2, 4, (2, 3)),
                    vt_slice(8, 12)(),
                ),
                lambda: (qT_dma(2)(), vt_slice(12, 16)()),  # k7; g2 at k=9
                lambda: kv_part("k", 4, 8, (0, 1)),      # k8; sb 16+ scores k=12
                lambda: kv_part("k", 4, 8, (2, 3)),      # k9
                lambda: kv_part("v", 4, 8, (0, 1)),      # k10
                lambda: (                                # k11; sb16 pops at k=14
                    kv_part("v", 4, 8, (2, 3)),
                    vt_slice(16, 20)(),
                ),
                lambda: (vt_slice(20, 26)(), epi_slot(0, 0)),  # k12
                lambda: (vt_slice(26, 32)(), epi_slot(0, 1)),  # k13
                lambda: epi_slot(0, 2),                  # k14
                lambda: epi_slot(0, 3),
                lambda: (epi_slot(1, 0), epi_slot(1, 1)),  # k16; g1 popped k=10
                lambda: (epi_slot(1, 2), epi_slot(1, 3)),
                None,                                    # k18
                lambda: epi_slot(3, 0),                  # k19; (3,12) popped k=18
                lambda: (                                # k20
                    epi_slot(3, 1),
                    epi_slot(2, 0),
                ),
            ]

            pending = []
            for k in range(len(B_items) + 1):
                if k < len(A_items):
                    pending.append(emit_scores(*A_items[k]))
                if 0 <= k - 1 < len(B_items):
                    pending.append(emit_scores(*B_items[k - 1]))
                if fillers:
                    f = fillers.pop(0)
                    if f is not None:
                        f()
                while len(pending) > 3:
                    n = 2 if len(pending) > 4 else 1
                    emit_items(pending[:n])
                    del pending[:n]
            # ---- tail: interleave the final pops with the remaining ----
            # g3/g2 epi slots, splitting copy/mul across ACT and DVE
            epi_slot(3, 2, copy_act=True)
            epi_slot(2, 1, mul_act=True)
            emit_items(pending[:2])
            del pending[:2]
            epi_slot(3, 3, copy_act=True)
            epi_slot(2, 2, mul_act=True)
            while pending:
                emit_items(pending[:1])
                del pending[:1]
            epi_slot(2, 3, copy_act=True, mul_act=True)

    nc.compile()
    return nc


def _get_nc():
    if "nc" not in _CACHE:
        _CACHE["nc"] = build_nc()
    return _CACHE["nc"]


def make_in_maps(x, Wq, bq, Wk, bk, Wv, bv, Wp, bp):
    """Build the 8 per-core input maps (host-side sharding)."""
    x = np.asarray(x, dtype=np.float32)
    tri = np.tril(np.ones((128, 128), dtype=np.float32)).T  # [s,t]: 1 iff s<=t
    wqkv_s = np.stack(
        [np.asarray(w, np.float32).reshape(4, 128, H) for w in (Wq, Wk, Wv)]
    )  # [3, 4(c), 128(p), H]
    wqkv_cols = np.transpose(wqkv_s, (2, 0, 1, 3)).reshape(128, 3 * 4 * H)
    wp_cols = np.tile(np.asarray(Wp, np.float32), (4, 1))  # [128, D]
    ident_cols = np.tile(np.eye(H, dtype=np.float32), (4, 1))  # [128, H]
    bqk_s = np.stack(
        [np.tile(np.asarray(b, np.float32), 4) for b in (bq, bk)], axis=1
    )
    bqk_s = np.ascontiguousarray(bqk_s, dtype=np.float32)

    in_maps = []
    for c in range(8):
        b, p = divmod(c, 2)
        xb = x[b]  # [T, D]
        if p == 1:
            xb = xb.reshape(T // 256, 2, 128, D)[:, ::-1].reshape(T, D)
        xT_c = np.ascontiguousarray(xb.T).astype(BF16).reshape(4, 128, T)
        if p == 0:
            m = np.stack([tri, np.zeros((128, 128), np.float32)])
        else:
            m = np.stack([tri, np.ones((128, 128), np.float32)])
        mask_cols = m.transpose(1, 0, 2).reshape(128, 2 * 128)
        maskadd_cols = (m - 1.0).transpose(1, 0, 2).reshape(128, 2 * 128) * 200.0
        pack = np.concatenate(
            [wqkv_cols, wp_cols, mask_cols, ident_cols, maskadd_cols], axis=1
        )
        in_maps.append(
            {
                "xT": xT_c,
                "pack": np.ascontiguousarray(pack).astype(BF16),
                "bqk": bqk_s,
            }
        )
    return in_maps


def assemble_out(results, bv, Wp, bp):
    """Gather per-core [2048, 512] bf16 outputs into [B, T, D] fp32."""
    out = np.empty((B, T, D), dtype=np.float32)
    for c in range(8):
        b, p = divmod(c, 2)
        oc = np.asarray(results[c]["out"]).astype(np.float32).reshape(
            NSLOT, 128, D
        )
        for j in range(NSLOT):
            g = 2 * j + p
            out[b, g * 128 : (g + 1) * 128, :] = oc[j]
    out += (
        np.asarray(bv, np.float32) @ np.asarray(Wp, np.float32)
        + np.asarray(bp, np.float32)
    )[None, None, :]
    return out


def run_axon_percore(nc, in_maps, n_cores=8):
    """Run the same single-core NEFF on n_cores axon devices.

    bass2jax.run_bass_via_pjrt's multi-core branch uses shard_map over
    an 8-device mesh; under the axon loopback relay that execution
    never completes. The kernel is pure data-parallel (no collectives),
    so n_cores independent per-device jit calls are semantically
    identical; jax's async dispatch lets them run concurrently.
    """
    import jax
    import concourse.mybir as mybir
    from concourse import bass2jax

    bass2jax.install_neuronx_cc_hook()

    partition_name = (
        nc.partition_id_tensor.name if nc.partition_id_tensor else None
    )
    in_names = []
    out_names = []
    out_avals = []
    zero_outs = []
    for alloc in nc.m.functions[0].allocations:
        if not isinstance(alloc, mybir.MemoryLocationSet):
            continue
        name = alloc.memorylocations[0].name
        if alloc.kind == "ExternalInput":
            if name != partition_name:
                in_names.append(name)
        elif alloc.kind == "ExternalOutput":
            out_names.append(name)
            shape = tuple(alloc.tensor_shape)
            dtype = mybir.dt.np(alloc.dtype)
            out_avals.append(jax.core.ShapedArray(shape, dtype))
            zero_outs.append(np.zeros(shape, dtype))
    n_params = len(in_names)
    all_names = in_names + out_names
    if partition_name is not None:
        all_names = all_names + [partition_name]

    def _body(*args):
        operands = list(args)
        if partition_name is not None:
            operands.append(bass2jax.partition_id_tensor())
        outs = bass2jax._bass_exec_p.bind(
            *operands,
            out_avals=tuple(out_avals),
            in_names=tuple(all_names),
            out_names=tuple(out_names),
            lowering_input_output_aliases=(),
            sim_require_finite=True,
            sim_require_nnan=True,
            nc=nc,
        )
        return tuple(outs)

    donate = tuple(range(n_params, n_params + len(out_names)))
    f = jax.jit(_body, donate_argnums=donate, keep_unused=True)
    devices = jax.devices()[:n_cores]
    # stage ALL inputs onto the devices before dispatching any kernel:
    # interleaving device_put with dispatch makes core 0 execute while
    # cores 1-7's host->HBM input transfers are still in flight (measured
    # ~5us slower on core 0 than core 7)
    all_args = []
    for c in range(n_cores):
        args = [
            jax.device_put(np.asarray(in_maps[c][k]), devices[c])
            for k in in_names
        ] + [jax.device_put(z, devices[c]) for z in zero_outs]
        all_args.append(args)
    for args in all_args:
        for a in args:
            a.block_until_ready()
    pending = [f(*args) for args in all_args]
    return [
        {name: np.asarray(outs[i]) for i, name in enumerate(out_names)}
        for outs in pending
    ]


def kernel(x, Wq, bq, Wk, bk, Wv, bv, Wp, bp):
    from concourse import bass_utils
    from concourse._compat import axon_active

    nc = _get_nc()
    in_maps = make_in_maps(x, Wq, bq, Wk, bk, Wv, bv, Wp, bp)
    if axon_active():
        results = run_axon_percore(nc, in_maps)
    else:
        res = bass_utils.run_bass_kernel_spmd(
            nc, in_maps, core_ids=list(range(8))
        )
        results = res.results
    return assemble_out(results, bv, Wp, bp)
